# revision 14
# baseline (speedup 1.0000x reference)
"""Trainium2 Bass kernel for nn_GPT_Network (4-layer windowed-attention GPT, B=2 S=1024
HID=512 FF=2048 NH=8 V=32000) on 8 NeuronCores.

Sharding (uniform SPMD, per-core behavior is data-driven):
  - tokens: core c owns 256 tokens (batch c//4, quarter c%4); residual stream +
    FF + attention queries are token-sharded; one 4MB AllGather of the residual
    stream per layer resynchronizes.
  - attention: each core computes K/V for its whole batch (1024 tokens, all heads)
    and windowed attention for its own 256 queries; window masks are host-built
    per-core inputs (multiplicative, post-exp).
  - decoder: vocab-sharded (4000 columns per core), host concatenates.
  - matmuls run as float32r (full PE rate, moving dim >= 256).
"""
import os
import sys

sys.path.insert(0, "/opt/trn_rl_repo")

import numpy as np

import concourse.bass as bass
import concourse.mybir as mybir
import concourse.tile as tile
from concourse import bacc
from concourse.bass_utils import run_bass_kernel_spmd
from concourse.masks import make_identity

F32 = mybir.dt.float32
F32R = mybir.dt.float32r
I32 = mybir.dt.int32
AF = mybir.ActivationFunctionType
ALU = mybir.AluOpType

# model dims (fixed by the problem)
L, NH, HID, FF, V, S, E, B = 4, 8, 512, 2048, 32000, 1024, 128, 2
HD = HID // NH  # 64
T = B * S  # 2048 tokens
WIN = [S // (z + 1) for z in range(L)][::-1]  # [256, 341, 512, 1024]
N_CORES = 8
OWN = T // N_CORES  # 256 own tokens per core
VSH = V // N_CORES  # 4000 vocab columns per core
VC = 500            # vocab chunk (8 chunks of 500)
P = 128
BT = S // P         # 8 batch tiles
OT = OWN // P       # 2 own tiles
KT = HID // P       # 4 hidden tiles
FT = FF // P        # 16 ff tiles
TT = T // P         # 16 total token tiles
EPS = 1e-6


def mmr(nc, out, lhsT, rhs, start, stop):
    nc.tensor.matmul(out, lhsT, rhs, start=start, stop=stop)


class KB:
    """Kernel builder: holds nc/tc/pools."""

    def __init__(self, skip_ln_affine, skip_ff_bias):
        self.skip_ln_affine = skip_ln_affine
        self.skip_ff_bias = skip_ff_bias
        nc = bacc.Bacc("TRN2", target_bir_lowering=False, debug=False,
                       num_devices=N_CORES)
        self.nc = nc
        # ---- I/O ----
        di = {}
        di["ids"] = nc.dram_tensor("ids", [T, 1], I32, kind="ExternalInput")
        di["W_emb"] = nc.dram_tensor("W_emb", [V, E], F32, kind="ExternalInput")
        di["W_dec_lin"] = nc.dram_tensor("W_dec_lin", [E, HID], F32, kind="ExternalInput")
        di["batch_idx"] = nc.dram_tensor("batch_idx", [S, 1], I32, kind="ExternalInput")
        di["own_idx"] = nc.dram_tensor("own_idx", [OWN, 1], I32, kind="ExternalInput")
        di["own_inb_idx"] = nc.dram_tensor("own_inb_idx", [OWN, 1], I32, kind="ExternalInput")
        for l in range(L):
            for w in ("Wq", "Wk", "Wv", "Wc"):
                di[f"{w}{l}"] = nc.dram_tensor(f"{w}{l}", [HID, HID], F32, kind="ExternalInput")
            di[f"ff1_{l}"] = nc.dram_tensor(f"ff1_{l}", [HID, FF], F32, kind="ExternalInput")
            di[f"ff2_{l}"] = nc.dram_tensor(f"ff2_{l}", [FF, HID], F32, kind="ExternalInput")
            di[f"pos{l}"] = nc.dram_tensor(f"pos{l}", [S, HID], F32, kind="ExternalInput")
            di[f"mask{l}"] = nc.dram_tensor(f"mask{l}", [S, OWN], F32, kind="ExternalInput")
            if not skip_ff_bias:
                di[f"b1_{l}"] = nc.dram_tensor(f"b1_{l}", [P, FT], F32, kind="ExternalInput")
                di[f"b2_{l}"] = nc.dram_tensor(f"b2_{l}", [1, HID], F32, kind="ExternalInput")
            if not skip_ln_affine:
                for nm in ("si", "bi", "s1", "b1n", "s2", "b2n"):
                    di[f"{nm}{l}"] = nc.dram_tensor(f"{nm}{l}", [1, HID], F32, kind="ExternalInput")
        if not skip_ln_affine:
            di["so"] = nc.dram_tensor("so", [1, HID], F32, kind="ExternalInput")
            di["bo"] = nc.dram_tensor("bo", [1, HID], F32, kind="ExternalInput")
        di["pdec"] = nc.dram_tensor("pdec", [HID, VSH], F32, kind="ExternalInput")
        self.out_sh = nc.dram_tensor("out_sh", [T, VSH], F32, kind="ExternalOutput")
        self.di = di
        # internal DRAM
        self.dec_in_buf = nc.dram_tensor("dec_in_buf", [T, HID], F32)
        self.ag_in = [nc.dram_tensor(f"ag_in{l}", [OWN, HID], F32) for l in range(L)]
        self.ag_out = [nc.dram_tensor(f"ag_out{l}", [T, HID], F32, addr_space="Shared")
                       for l in range(L)]
        self._evict_ctr = 0

    def evict_copy(self, out, in_):
        # alternate PSUM->SBUF copy engine between DVE and ACT
        self._evict_ctr += 1
        if self._evict_ctr % 2:
            self.nc.vector.tensor_copy(out=out, in_=in_)
        else:
            self.nc.scalar.copy(out=out, in_=in_)

    # ---------------- building blocks ----------------
    def ln(self, out_ap, in_ap, stats_pool, scale_ap=None, bias_ap=None,
           extra_add=None):
        """out = LN(in)*scale + bias (+ extra_add). in/out [128, 512] SBUF."""
        nc = self.nc
        st = stats_pool.tile([P, 6], F32, tag="ln_stats")
        mv = stats_pool.tile([P, 2], F32, tag="ln_mv")
        nc.vector.bn_stats(out=st[:], in_=in_ap)
        nc.vector.bn_aggr(out=mv[:], in_=st[:])
        # mv[:,1] = 1/sqrt(var+eps)
        nc.scalar.activation(out=mv[:, 1:2], in_=mv[:, 1:2], func=AF.Sqrt,
                             bias=self.eps_sb[:])
        nc.vector.reciprocal(out=mv[:, 1:2], in_=mv[:, 1:2])
        nc.vector.tensor_scalar(out=out_ap, in0=in_ap, scalar1=mv[:, 0:1],
                                scalar2=mv[:, 1:2], op0=ALU.subtract, op1=ALU.mult)
        if scale_ap is not None:
            nc.gpsimd.tensor_tensor(out=out_ap, in0=out_ap, in1=scale_ap, op=ALU.mult)
        if bias_ap is not None:
            nc.gpsimd.tensor_tensor(out=out_ap, in0=out_ap, in1=bias_ap, op=ALU.add)
        if extra_add is not None:
            nc.vector.tensor_tensor(out=out_ap, in0=out_ap, in1=extra_add, op=ALU.add)

    def transpose128(self, pools, out_sb_ap, in_sb_ap):
        """out[128,128] = in^T via PE."""
        nc = self.nc
        pt = pools["ps_t"].tile([P, P], F32, tag="tp")
        nc.tensor.transpose(out=pt[:], in_=in_sb_ap, identity=self.ident[:])
        self.evict_copy(out=out_sb_ap, in_=pt[:])

    def build(self):
        nc = self.nc
        with nc.allow_low_precision(reason="fp32r matmul inputs; fp32 accumulation throughout"):
            with tile.TileContext(nc) as tc:
                self.tc = tc
                self._build_inner(tc)
        nc.compile()
        return nc

    def _build_inner(self, tc):
        nc = self.nc
        di = self.di
        from contextlib import ExitStack
        ctx = ExitStack()
        self._ctx = ctx
        const = ctx.enter_context(tc.tile_pool(name="const", bufs=1))
        stats = ctx.enter_context(tc.tile_pool(name="stats", bufs=4))
        big = ctx.enter_context(tc.tile_pool(name="big", bufs=1))
        small = ctx.enter_context(tc.tile_pool(name="small", bufs=2))
        stream = ctx.enter_context(tc.tile_pool(name="stream", bufs=2))
        stream4 = ctx.enter_context(tc.tile_pool(name="stream4", bufs=4))
        wpool = ctx.enter_context(tc.tile_pool(name="wpool", bufs=1))
        ps_mm = ctx.enter_context(tc.tile_pool(name="ps_mm", bufs=2, space="PSUM"))
        ps_s = ctx.enter_context(tc.tile_pool(name="ps_s", bufs=2, space="PSUM"))
        ps_ctx = ctx.enter_context(tc.tile_pool(name="ps_ctx", bufs=2, space="PSUM"))
        ps_t = ctx.enter_context(tc.tile_pool(name="ps_t", bufs=2, space="PSUM"))
        pools = dict(ps_mm=ps_mm, ps_s=ps_s, ps_ctx=ps_ctx, ps_t=ps_t,
                     stream=stream, stream4=stream4, small=small, stats=stats)

        # constants
        self.ident = const.tile([P, P], F32)
        make_identity(nc, self.ident)
        ones_f = const.tile([1, P], F32)
        nc.vector.memset(ones_f[:], 1.0)
        ones_m = const.tile([1, P], F32R)  # lhsT for bias rows / den bcast
        nc.vector.tensor_copy(out=ones_m[:], in_=ones_f[:])
        ones_blk = const.tile([P, BT, NH], F32)
        nc.vector.memset(ones_blk[:], 1.0)
        self.ones_blk = ones_blk
        self.eps_sb = const.tile([P, 1], F32)
        nc.vector.memset(self.eps_sb[:], EPS)

        ids_sb = const.tile([P, TT], I32)
        nc.sync.dma_start(out=ids_sb[:], in_=di["ids"].ap().rearrange("(t p) one -> p (t one)", p=P))
        bidx_sb = const.tile([P, BT], I32)
        nc.sync.dma_start(out=bidx_sb[:], in_=di["batch_idx"].ap().rearrange("(t p) one -> p (t one)", p=P))
        oidx_sb = const.tile([P, OT], I32)
        nc.sync.dma_start(out=oidx_sb[:], in_=di["own_idx"].ap().rearrange("(t p) one -> p (t one)", p=P))
        oinb_sb = const.tile([P, OT], I32)
        nc.sync.dma_start(out=oinb_sb[:], in_=di["own_inb_idx"].ap().rearrange("(t p) one -> p (t one)", p=P))

        def bcast_row(name):
            t = small.tile([P, HID], F32, tag="lnp_" + name[:2])
            src = di[name].ap()
            bc = bass.AP(tensor=src.tensor, offset=src.offset, ap=[[0, P], [1, HID]])
            nc.sync.dma_start(out=t[:], in_=bc)
            return t

        # ---------------- embedding (replicated) ----------------
        wdl = const.tile([P, HID], F32R)
        nc.sync.dma_start(out=wdl[:], in_=di["W_dec_lin"][:, :].bitcast(F32R))
        with nc.named_scope("embed"):
            for t in range(TT):
                emb = stream.tile([P, E], F32, tag="emb")
                nc.gpsimd.indirect_dma_start(
                    out=emb[:], out_offset=None, in_=di["W_emb"][:, :],
                    in_offset=bass.IndirectOffsetOnAxis(ap=ids_sb[:, t:t + 1], axis=0))
                embT = stream.tile([P, E], F32R, tag="embT")
                self.transpose128(pools, embT[:], emb[:])
                pm = ps_mm.tile([P, HID], F32, tag="mm")
                mmr(nc, pm[:], embT[:], wdl[:], True, True)
                dtile = stream.tile([P, HID], F32, tag="dec_in")
                self.evict_copy(out=dtile[:], in_=pm[:])
                nc.sync.dma_start(out=self.dec_in_buf[t * P:(t + 1) * P, :], in_=dtile[:])

        # ---------------- layers ----------------
        for l in range(L):
            src = self.dec_in_buf if l == 0 else self.ag_out[l - 1]
            with nc.named_scope(f"layer{l}"):
                self._build_layer(l, src, pools, big, const, ones_m, bidx_sb,
                                  oidx_sb, oinb_sb, bcast_row, wpool)

        # ---------------- final LN + decoder ----------------
        with nc.named_scope("final"):
            so = bcast_row("so") if not self.skip_ln_affine else None
            bo = bcast_row("bo") if not self.skip_ln_affine else None
            doT = big.tile([P, KT, T], F32R, tag="kT")  # reuse kT slot (dead)
            for t in range(TT):
                xf = stream.tile([P, HID], F32, tag="xf")
                nc.sync.dma_start(out=xf[:], in_=self.ag_out[L - 1][t * P:(t + 1) * P, :])
                dint = stream.tile([P, HID], F32, tag="dint")
                nc.sync.dma_start(out=dint[:], in_=self.dec_in_buf[t * P:(t + 1) * P, :])
                dout = stream.tile([P, HID], F32, tag="dout")
                self.ln(dout[:], xf[:], stats,
                        so[:] if so is not None else None,
                        bo[:] if bo is not None else None, extra_add=dint[:])
                for f in range(KT):
                    self.transpose128(pools, doT[:, f, t * P:(t + 1) * P],
                                      dout[:, f * P:(f + 1) * P])
            with nc.named_scope("decoder"):
                for n in range(VSH // VC):
                    pd = []
                    for k in range(KT):
                        pdt = stream4.tile([P, VC], F32R, tag="pdec")
                        nc.sync.dma_start(
                            out=pdt[:], in_=di["pdec"][k * P:(k + 1) * P, n * VC:(n + 1) * VC].bitcast(F32R))
                        pd.append(pdt)
                    for mt in range(TT):
                        pm = ps_mm.tile([P, VC], F32, tag="mm")
                        for k in range(KT):
                            mmr(nc, pm[:], doT[:, k, mt * P:(mt + 1) * P], pd[k][:],
                                k == 0, k == KT - 1)
                        ot = stream4.tile([P, VC], F32, tag="osb")
                        self.evict_copy(out=ot[:], in_=pm[:])
                        nc.sync.dma_start(
                            out=self.out_sh[mt * P:(mt + 1) * P, n * VC:(n + 1) * VC],
                            in_=ot[:])

        ctx.close()

    def _build_layer(self, l, src, pools, big, const, ones_m, bidx_sb, oidx_sb,
                     oinb_sb, bcast_row, wpool):
        nc = self.nc
        di = self.di
        tc = self.tc
        stream = pools["stream"]
        stats = pools["stats"]
        ps_mm = pools["ps_mm"]
        ps_s = pools["ps_s"]
        ps_ctx = pools["ps_ctx"]
        ps_t = pools["ps_t"]

        if not self.skip_ln_affine:
            si = bcast_row(f"si{l}")
            bi = bcast_row(f"bi{l}")
        else:
            si = bi = None

        # -- gather x batch rows + LN + pos -> lin_b [128, 8, 512]
        lin_b = big.tile([P, BT, HID], F32, tag="lin_b")
        for j in range(BT):
            xb = stream.tile([P, HID], F32, tag="xb")
            nc.gpsimd.indirect_dma_start(
                out=xb[:], out_offset=None, in_=src[:, :],
                in_offset=bass.IndirectOffsetOnAxis(ap=bidx_sb[:, j:j + 1], axis=0))
            post = stream.tile([P, HID], F32, tag="pos")
            nc.sync.dma_start(out=post[:], in_=di[f"pos{l}"][j * P:(j + 1) * P, :])
            self.ln(lin_b[:, j, :], xb[:], stats,
                    si[:] if si is not None else None,
                    bi[:] if bi is not None else None, extra_add=post[:])
        # -- own rows
        lin_o = big.tile([P, OT, HID], F32, tag="lin_o")
        for j in range(OT):
            xo = stream.tile([P, HID], F32, tag="xo")
            nc.gpsimd.indirect_dma_start(
                out=xo[:], out_offset=None, in_=src[:, :],
                in_offset=bass.IndirectOffsetOnAxis(ap=oidx_sb[:, j:j + 1], axis=0))
            poso = stream.tile([P, HID], F32, tag="poso")
            nc.gpsimd.indirect_dma_start(
                out=poso[:], out_offset=None, in_=di[f"pos{l}"][:, :],
                in_offset=bass.IndirectOffsetOnAxis(ap=oinb_sb[:, j:j + 1], axis=0))
            self.ln(lin_o[:, j, :], xo[:], stats,
                    si[:] if si is not None else None,
                    bi[:] if bi is not None else None, extra_add=poso[:])

        # -- transposes: linT [128, 4, 1024] (features x batch tokens), ownT [128,4,256]
        linT = big.tile([P, KT, S], F32R, tag="linT")
        for j in range(BT):
            for f in range(KT):
                self.transpose128(pools, linT[:, f, j * P:(j + 1) * P],
                                  lin_b[:, j, f * P:(f + 1) * P])
        ownT = big.tile([P, KT, OWN], F32R, tag="ownT")
        for j in range(OT):
            for f in range(KT):
                self.transpose128(pools, ownT[:, f, j * P:(j + 1) * P],
                                  lin_o[:, j, f * P:(f + 1) * P])

        # -- K^T [128, 4, 1024]
        def load_w(name):
            w = wpool.tile([P, KT, HID], F32R, tag="w4")
            for k in range(KT):
                nc.sync.dma_start(out=w[:, k, :],
                                  in_=di[name][k * P:(k + 1) * P, :].bitcast(F32R))
            return w

        kTt = big.tile([P, KT, S], F32R, tag="kT")
        wk = load_w(f"Wk{l}")
        for m in range(KT):
            for n in range(S // 512):
                pm = ps_mm.tile([P, 512], F32, tag="mm")
                for k in range(KT):
                    mmr(nc, pm[:], wk[:, k, m * P:(m + 1) * P],
                        linT[:, k, n * 512:(n + 1) * 512], k == 0, k == KT - 1)
                self.evict_copy(out=kTt[:, m, n * 512:(n + 1) * 512], in_=pm[:])
        # -- V [128, 8, 512] then augmented va [128, 8, 65] per head
        vt = big.tile([P, BT, HID], F32, tag="lin_b")  # reuse lin_b slot (dead)
        wv = load_w(f"Wv{l}")
        for mt in range(BT):
            pm = ps_mm.tile([P, 512], F32, tag="mm")
            for k in range(KT):
                mmr(nc, pm[:], linT[:, k, mt * P:(mt + 1) * P], wv[:, k, :],
                    k == 0, k == KT - 1)
            self.evict_copy(out=vt[:, mt, :], in_=pm[:])
        va = big.tile([P, BT, NH, HD + 1], F32R, tag="va")
        nc.gpsimd.tensor_copy(out=va[:, :, :, HD], in_=self.ones_blk[:])  # ones col
        for mt in range(BT):
            nc.vector.tensor_copy(
                out=va[:, mt, :, 0:HD],
                in_=vt[:, mt, :].rearrange("p (h d) -> p h d", h=NH))
        # -- q^T [128, 4, 256] scaled by 1/8
        qTt = big.tile([P, KT, OWN], F32R, tag="qT")
        wq = load_w(f"Wq{l}")
        for m in range(KT):
            pm = ps_mm.tile([P, OWN], F32, tag="mm")
            for k in range(KT):
                mmr(nc, pm[:], wq[:, k, m * P:(m + 1) * P], ownT[:, k, :],
                    k == 0, k == KT - 1)
            nc.scalar.activation(out=qTt[:, m, :], in_=pm[:], func=AF.Copy,
                                 scale=1.0 / np.sqrt(HD))
        # -- masks [128, 8, 256]
        mk = big.tile([P, BT, OWN], F32R, tag="ownT")  # reuse ownT slot (dead)
        for j in range(BT):
            nc.sync.dma_start(out=mk[:, j, :], in_=di[f"mask{l}"][j * P:(j + 1) * P, :].bitcast(F32R))

        # -- attention: per head, accumulate ctx^T (+den) over 8 key chunks
        ctxT = big.tile([P, KT, OWN], F32R, tag="ctxT")
        for h in range(NH):
            kt_i, kt_o = h // 2, (h % 2) * HD
            pc = ps_ctx.tile([HD + 1, OWN], F32, tag="ctx")
            for j in range(BT):
                ps = ps_s.tile([P, OWN], F32, tag="sc")
                mmr(nc, ps[:], kTt[:, kt_i, j * P:(j + 1) * P][kt_o:kt_o + HD, :],
                    qTt[:, kt_i, :][kt_o:kt_o + HD, :], True, True)
                ex = pools["stream4"].tile([P, OWN], F32R, tag="exp")
                nc.scalar.activation(out=ex[:], in_=ps[:], func=AF.Exp)
                nc.vector.tensor_tensor(out=ex[:], in0=ex[:], in1=mk[:, j, :], op=ALU.mult)
                mmr(nc, pc[:], va[:, j, h, :], ex[:], j == 0, j == BT - 1)
            craw = pools["stream"].tile([HD, OWN], F32, tag="craw")
            nc.scalar.copy(out=craw[:], in_=pc[0:HD, :])
            rec = stats.tile([1, OWN], F32R, tag="rec")
            nc.vector.reciprocal(out=rec[:], in_=pc[HD:HD + 1, :])
            pb = ps_s.tile([HD, OWN], F32, tag="sc")  # share scores psum slots
            mmr(nc, pb[:], ones_m[:, 0:HD], rec[:], True, True)
            nc.vector.tensor_tensor(
                out=ctxT[:, kt_i, :][kt_o:kt_o + HD, :], in0=craw[:], in1=pb[:],
                op=ALU.mult)

        # -- attn_out (own tokens) = ctx @ Wc : [128, 2, 512]
        wc = load_w(f"Wc{l}")
        ao = big.tile([P, OT, HID], F32, tag="ao")
        for mt in range(OT):
            pm = ps_mm.tile([P, 512], F32, tag="mm")
            for k in range(KT):
                mmr(nc, pm[:], ctxT[:, k, mt * P:(mt + 1) * P], wc[:, k, :],
                    k == 0, k == KT - 1)
            self.evict_copy(out=ao[:, mt, :], in_=pm[:])

        # -- x1 = lin_o + LN(ao); x1T
        if not self.skip_ln_affine:
            s1 = bcast_row(f"s1{l}")
            b1n = bcast_row(f"b1n{l}")
        else:
            s1 = b1n = None
        x1 = big.tile([P, OT, HID], F32, tag="x1")
        for j in range(OT):
            self.ln(x1[:, j, :], ao[:, j, :], stats,
                    s1[:] if s1 is not None else None,
                    b1n[:] if b1n is not None else None, extra_add=lin_o[:, j, :])
        x1T = big.tile([P, KT, OWN], F32R, tag="ctxT")  # reuse ctxT slot (dead)
        for j in range(OT):
            for f in range(KT):
                self.transpose128(pools, x1T[:, f, j * P:(j + 1) * P],
                                  x1[:, j, f * P:(f + 1) * P])

        # -- ff1 (choice-2): hT [128, 16, 256] = relu(ff1^T x1^T + b1)
        if not self.skip_ff_bias:
            b1sb = pools["small"].tile([P, FT], F32, tag="b1sb")
            nc.sync.dma_start(out=b1sb[:], in_=di[f"b1_{l}"][:, :])
        hT = big.tile([P, FT, OWN], F32R, tag="linT")  # reuse linT slot (dead)
        for m in range(FT):
            f1 = pools["stream4"].tile([P, KT, P], F32R, tag="f1w")
            for k in range(KT):
                nc.sync.dma_start(out=f1[:, k, :],
                                  in_=di[f"ff1_{l}"][k * P:(k + 1) * P, m * P:(m + 1) * P].bitcast(F32R))
            pm = ps_mm.tile([P, OWN], F32, tag="mm")
            for k in range(KT):
                mmr(nc, pm[:], f1[:, k, :], x1T[:, k, :], k == 0, k == KT - 1)
            if self.skip_ff_bias:
                nc.scalar.activation(out=hT[:, m, :], in_=pm[:], func=AF.Relu)
            else:
                nc.scalar.activation(out=hT[:, m, :], in_=pm[:], func=AF.Relu,
                                     bias=b1sb[:, m:m + 1])

        # -- ff2 (choice-1): ffw [128, 2, 512] = h @ ff2 (+ b2 via ones-row)
        if not self.skip_ff_bias:
            b2sb = pools["small"].tile([1, HID], F32R, tag="b2sb")
            nc.sync.dma_start(out=b2sb[:], in_=di[f"b2_{l}"][:, :].bitcast(F32R))
        ffw = big.tile([P, OT, HID], F32, tag="ao")  # reuse ao slot (dead)
        pm2 = []
        for _i in range(OT):
            pm2t = ps_mm.tile([P, 512], F32, tag="mm")
            pm2.append(pm2t)
        for k in range(FT):
            f2 = pools["stream4"].tile([P, HID], F32R, tag="f2w")
            nc.sync.dma_start(out=f2[:], in_=di[f"ff2_{l}"][k * P:(k + 1) * P, :].bitcast(F32R))
            last = (k == FT - 1) and self.skip_ff_bias
            for mt in range(OT):
                mmr(nc, pm2[mt][:], hT[:, k, mt * P:(mt + 1) * P], f2[:],
                    k == 0, last)
        for mt in range(OT):
            if not self.skip_ff_bias:
                # += b2 broadcast over tokens (k=1 outer-product matmul)
                mmr(nc, pm2[mt][:], ones_m[0:1, 0:P], b2sb[:], False, True)
            self.evict_copy(out=ffw[:, mt, :], in_=pm2[mt][:])

        # -- x2 = x1 + LN(ffw) -> ag
        if not self.skip_ln_affine:
            s2 = bcast_row(f"s2{l}")
            b2n = bcast_row(f"b2n{l}")
        else:
            s2 = b2n = None
        x2 = big.tile([P, OT, HID], F32, tag="lin_o")  # reuse lin_o slot (dead)
        for j in range(OT):
            self.ln(x2[:, j, :], ffw[:, j, :], stats,
                    s2[:] if s2 is not None else None,
                    b2n[:] if b2n is not None else None, extra_add=x1[:, j, :])
            nc.sync.dma_start(out=self.ag_in[l][j * P:(j + 1) * P, :], in_=x2[:, j, :])
        nc.gpsimd.collective_compute(
            "AllGather", ALU.bypass, replica_groups=[list(range(N_CORES))],
            ins=[self.ag_in[l][:].opt()], outs=[self.ag_out[l][:].opt()])


_CACHE = {}


def _get_nc(skip_ln_affine, skip_ff_bias):
    key = (skip_ln_affine, skip_ff_bias)
    if key not in _CACHE:
        kb = KB(skip_ln_affine, skip_ff_bias)
        kb.build()
        _CACHE[key] = kb
    return _CACHE[key]


def _prep_inputs(inputs, skip_ln_affine, skip_ff_bias):
    """Build the 8 per-core input maps from the full inputs."""
    f32 = lambda a: np.ascontiguousarray(np.asarray(a), dtype=np.float32)
    x_input = np.asarray(inputs["x_input"])
    ids = np.ascontiguousarray(x_input.reshape(T, 1).astype(np.int32))
    base = {
        "ids": ids,
        "W_emb": f32(inputs["W_emb_dec"]),
        "W_dec_lin": f32(inputs["W_dec_lin"]),
    }
    for l in range(L):
        base[f"Wq{l}"] = f32(inputs["p_d_q"][l])
        base[f"Wk{l}"] = f32(inputs["p_d_k"][l])
        base[f"Wv{l}"] = f32(inputs["p_d_v"][l])
        base[f"Wc{l}"] = f32(inputs["p_d_c"][l])
        base[f"ff1_{l}"] = f32(inputs["p_d_ff1"][l])
        base[f"ff2_{l}"] = f32(inputs["p_d_ff2"][l])
        base[f"pos{l}"] = f32(inputs["x_emb_pos_dec"][l])
        if not skip_ff_bias:
            base[f"b1_{l}"] = f32(inputs["b_d_ff1"][l]).reshape(FT, P).T.copy()
            base[f"b2_{l}"] = f32(inputs["b_d_ff2"][l]).reshape(1, HID)
        if not skip_ln_affine:
            base[f"si{l}"] = f32(inputs["b_d_scale_i"][l]).reshape(1, HID)
            base[f"bi{l}"] = f32(inputs["b_d_bias_i"][l]).reshape(1, HID)
            base[f"s1{l}"] = f32(inputs["b_d_scale_1"][l]).reshape(1, HID)
            base[f"b1n{l}"] = f32(inputs["b_d_bias_1"][l]).reshape(1, HID)
            base[f"s2{l}"] = f32(inputs["b_d_scale_2"][l]).reshape(1, HID)
            base[f"b2n{l}"] = f32(inputs["b_d_bias_2"][l]).reshape(1, HID)
    if not skip_ln_affine:
        base["so"] = f32(inputs["d_o_scale"]).reshape(1, HID)
        base["bo"] = f32(inputs["d_o_bias"]).reshape(1, HID)
    pdec = f32(inputs["p_decoder"])

    in_maps = []
    ar = np.arange(S, dtype=np.int32)
    for c in range(N_CORES):
        b, q = divmod(c, 4)
        m = dict(base)
        m["batch_idx"] = np.ascontiguousarray((b * S + ar).reshape(S, 1))
        own_inb = q * OWN + np.arange(OWN, dtype=np.int32)
        m["own_idx"] = np.ascontiguousarray((b * S + own_inb).reshape(OWN, 1))
        m["own_inb_idx"] = np.ascontiguousarray(own_inb.reshape(OWN, 1))
        qg = (q * OWN + np.arange(OWN))[None, :]    # [1, 256] query pos
        kg = np.arange(S)[:, None]                  # [1024, 1] key pos
        for l in range(L):
            d = qg - kg
            m[f"mask{l}"] = np.ascontiguousarray(
                ((d >= 0) & (d <= WIN[l])).astype(np.float32))
        m["pdec"] = np.ascontiguousarray(pdec[:, c * VSH:(c + 1) * VSH])
        in_maps.append(m)
    return in_maps


def _assemble(results):
    full = np.concatenate([results[c]["out_sh"] for c in range(N_CORES)], axis=1)
    return full.reshape(B, S, V)


def run(inputs, trace=False):
    skip_ln_affine = all(
        np.allclose(inputs[k], 1.0 if "scale" in k else 0.0)
        for k in ("b_d_scale_i", "b_d_bias_i", "b_d_scale_1", "b_d_bias_1",
                  "b_d_scale_2", "b_d_bias_2", "d_o_scale", "d_o_bias"))
    skip_ff_bias = (np.allclose(inputs["b_d_ff1"], 0.0)
                    and np.allclose(inputs["b_d_ff2"], 0.0))
    kb = _get_nc(skip_ln_affine, skip_ff_bias)
    in_maps = _prep_inputs(inputs, skip_ln_affine, skip_ff_bias)
    res = run_bass_kernel_spmd(kb.nc, in_maps, list(range(N_CORES)), trace=trace)
    return _assemble(res.results), res


def kernel(**inputs) -> np.ndarray:
    out, _ = run(inputs, trace=False)
    return out
